# revision 25
# baseline (speedup 1.0000x reference)
"""AttentionNCF Trainium2 kernel (SPMD over 8 NeuronCores, data-parallel over B).

Math (per batch row b, rated item i):
  e_c = cand @ We.T + be                  [B, E]
  e_r = rated @ We.T + be                 [I, E]
  scores[b,i] = sum_a wa2[a] * relu(cp[b,a] + rp[i,a])
              = sum_a sign(wa2[a]) * relu(|wa2[a]|*(cp[b,a] + rp[i,a]))
  att = softmax_i(scores); user_emb = (att*um) @ e_r
  out = MLP(concat[e_c, user_emb])

Device mapping (per core, BC=1024 rows of B):
  H layout: partitions = (i_local 8, a 16), free = b.  |wa2| is folded into
  cp/rp on host so formation is a plain relu and the score contraction
  stationary is the exact sign pattern (+-1).
  Per 16-group chunk: 12 groups formed on DVE (bf16, 4x mode) -> bf16
  matmuls; 4 groups formed on ScalarE (fp8e4 out) -> 2 fp8 DoubleRow pair
  matmuls (2 k-tiles per instruction).  att = exp(scores + SHIFT) in fp8;
  aw = att*um in fp8 on GPSIMD; user_emb/denominator accumulate via fp8
  DoubleRow matmuls over chunk pairs.
"""

import sys

import ml_dtypes
import numpy as np

sys.path.insert(0, "/opt/trn_rl_repo")

BF = ml_dtypes.bfloat16
E4 = ml_dtypes.float8_e4m3

import concourse.bass as bass
import concourse.mybir as mybir
import concourse.tile as tile
from concourse import bacc
from concourse.bass_utils import run_bass_kernel_spmd

F32 = mybir.dt.float32
BF16 = mybir.dt.bfloat16
FP8 = mybir.dt.float8e4
AF = mybir.ActivationFunctionType
ALU = mybir.AluOpType
DR = mybir.MatmulPerfMode.DoubleRow

B, I, D, E, ATT = 8192, 1000, 1000, 64, 16
D1, D2 = 64, 32
NCORES = 8
BC = B // NCORES  # 1024 batch rows per core
NT = 8  # i-chunks of 128 (7 full + 1 partial of 104)
NPAD = 24  # pad i rows in chunk 7
SHIFT = 2.5  # softmax-invariant shift so exp() fits fp8e4 comfortably
PAD_ATT = float(np.asarray(np.exp(SHIFT), dtype=E4))  # fp8-rounded exp(SHIFT)
LNBIAS = -NPAD * PAD_ATT  # subtract pad contribution from the denominator

# per-chunk group split: first NS groups on DVE (bf16), then NPAIR fp8 pairs
# formed on ScalarE.  chunk 7 has 13 groups: 9 singles + pairs (9,10),(11,12).
NS_FULL, NPAIR = 12, 2
PAIR_SLOTS = [(12, 13), (14, 15), (9, 10), (11, 12)]  # w2p bank layout


def _chunk_plan(t):
    if t < NT - 1:
        return 12, [(0, (12, 13)), (1, (14, 15))]
    return 9, [(2, (9, 10)), (3, (11, 12))]


def emit(nc, io):
    """Emit the kernel body. io maps tensor names to DRAM APs."""
    with tile.TileContext(nc) as tc:
        with (
            tc.tile_pool(name="const", bufs=1) as cpool,
            tc.tile_pool(name="inbig", bufs=1) as ipool,
            tc.tile_pool(name="stat", bufs=1) as spool,
            tc.tile_pool(name="hform", bufs=11) as hpool,
            tc.tile_pool(name="hform8", bufs=3) as h8pool,
            tc.tile_pool(name="att", bufs=3) as apool,
            tc.tile_pool(name="aw", bufs=3) as awpool,
            tc.tile_pool(name="fin", bufs=2) as fpool,
            tc.tile_pool(name="pstmp", bufs=2, space="PSUM") as pstmp,
            tc.tile_pool(name="pssc", bufs=2, space="PSUM") as pssc,
            tc.tile_pool(name="pssu", bufs=1, space="PSUM") as pssu,
        ):
            # -------- inputs to SBUF (single sync DMA queue, needed-first) --
            cpack = cpool.tile([128, 260], F32)
            nc.sync.dma_start(out=cpack[:], in_=io["cpack"][:])
            cprep = spool.tile([128, BC], BF16)
            nc.sync.dma_start(out=cprep[:], in_=io["cprep"][:])
            rp_cols = cpack[:, 0:125]
            bm1_c = cpack[0:D1, 125:126]
            bm2_c = cpack[0:D2, 126:127]
            bm3_c = cpack[0:1, 127:128]
            ident = cpack[0:64, 128:192]
            shift_c = cpack[:, 192:193]
            neg1_c = cpack[0:1, 193:194]
            lnb_c = cpack[64:65, 193:194]
            ones_f32 = cpack[0:1, 194:258]
            wpk = cpool.tile([128, 1764], BF16)
            nc.sync.dma_start(out=wpk[:, 0:256], in_=io["wpk"][:, 0:256])
            nc.sync.dma_start(out=wpk[:, 256:1764], in_=io["wpk"][:, 256:1764])
            wm1aT = wpk[0:E, 1536:1600]
            wm1bT = wpk[0:E, 1600:1664]
            wm2T = wpk[0:D1, 1664:1696]
            wm3T = wpk[0:D2, 1696:1697]
            ones_bf = wpk[0:1, 1697:1761]
            w2p = cpool.tile([128, 4, 2, 128], FP8)
            nc.sync.dma_start(out=w2p[:], in_=io["w2p"][:])
            weT = cpool.tile([128, NT, E], BF16)
            nc.sync.dma_start(out=weT[:], in_=io["weT"][:])
            weT8 = cpool.tile([128, NT, E], FP8)
            nc.sync.dma_start(out=weT8[:], in_=io["weT8"][:])
            rated8 = ipool.tile([128, NT, I], FP8)
            nc.sync.dma_start(out=rated8[:], in_=io["ratedT"][:])
            ums = []
            for u in range(4):
                um_t = ipool.tile([128, 2, BC], FP8, name=f"um{u}")
                ums.append(um_t)
            nc.sync.dma_start(out=ums[0][:], in_=io["um0"][:])
            cand = ipool.tile([128, NT, BC], BF16)
            nc.sync.dma_start(out=cand[:], in_=io["candT"][:])
            for u in range(1, 4):
                nc.sync.dma_start(out=ums[u][:], in_=io[f"um{u}"][:])

            ones32 = cpool.tile([128, 2, 32], FP8)
            nc.gpsimd.memset(ones32[:], 1.0)

            # act-table warmup: touch Relu/Exp/Ln before any real ACT work so
            # the table load(s) happen during the initial DMA wait, not on the
            # critical path.
            warm = cpool.tile([1, 4], F32)
            nc.vector.memset(warm[:], 1.0)
            wout = cpool.tile([1, 4], F32)
            nc.scalar.activation(wout[:], warm[:], AF.Relu)
            nc.scalar.activation(wout[:], warm[:], AF.Exp)

            e_cT = spool.tile([E, BC], BF16)
            e_r8 = spool.tile([128, NT, E], FP8)

            def emit_ecT():
                for h in range(2):
                    sl = slice(512 * h, 512 * (h + 1))
                    ps = pstmp.tile([128, 512], F32, tag="tmp", name=f"psec{h}")
                    for c in range(NT):
                        nc.tensor.matmul(
                            ps[:E, :], weT[:, c, :], cand[:, c, sl],
                            start=(c == 0), stop=(c == NT - 1),
                        )
                    nc.vector.tensor_copy(e_cT[:, sl], ps[:E, :])

            def emit_er_setup():
                e_rT = spool.tile([E, 1024], F32)
                nc.vector.memset(e_rT[:, I:1024], 0.0)
                for h, n0, nw in ((0, 0, 500), (1, 500, 500)):
                    ps = pstmp.tile([128, 512], F32, tag="tmp")
                    for c2 in range(NT // 2):
                        nc.tensor.matmul(
                            ps[:E, :nw],
                            weT8[:, 2 * c2 : 2 * c2 + 2, :],
                            rated8[:, 2 * c2 : 2 * c2 + 2, n0 : n0 + nw],
                            start=(c2 == 0), stop=(c2 == NT // 2 - 1), perf_mode=DR,
                        )
                    nc.vector.tensor_copy(e_rT[:, n0 : n0 + nw], ps[:E, :nw])
                for c in range(NT):
                    ps = pstmp.tile([128, 512], F32, tag="tmp")
                    nc.tensor.transpose(ps[:, :E], e_rT[:, 128 * c : 128 * (c + 1)], ident)
                    nc.vector.tensor_copy(e_r8[:, c, :], ps[:, :E])

            # ---------------- main loop over i-chunks ----------------------
            su0 = pssu.tile([64, 512], F32)  # user_emb accumulators
            su1 = pssu.tile([64, 512], F32)
            sus = (su0, su1)
            dns = [None, None]  # denominator accumulators (pstmp bufs, DR dst 0)
            scs = [None] * NT
            atts = [None] * (NT // 2)  # per chunk pair
            aws = [None] * (NT // 2)

            def emit_chunk(t, after_first_pair=None, half_major=False):
                ns, prs = _chunk_plan(t)
                sc = pssc.tile([128, 1024], F32, tag="sc")
                hTs, hPs = [], []
                for g in range(ns):
                    G = 16 * t + g
                    hT = hpool.tile([128, BC], BF16, tag="h", name=f"hT{t}_{g}")
                    nc.vector.tensor_scalar(
                        hT[:], cprep[:], rp_cols[:, G : G + 1], 0.0, ALU.add, ALU.max
                    )
                    hTs.append(hT)
                    if not half_major:
                        for h in range(2):
                            nc.tensor.matmul(
                                sc[:, 512 * h : 512 * (h + 1)],
                                wpk[:, 128 * g : 128 * (g + 1)],
                                hT[:, 512 * h : 512 * (h + 1)],
                                start=(g == 0), stop=False,
                            )
                for k, (slot, (gA, gB)) in enumerate(prs):
                    hP = h8pool.tile([128, 2, BC], FP8, tag="h8", name=f"hP{t}_{k}")
                    for s, g in ((0, gA), (1, gB)):
                        G = 16 * t + g
                        nc.scalar.activation(
                            hP[:, s, :], cprep[:], AF.Relu, bias=rp_cols[:, G : G + 1]
                        )
                    hPs.append((slot, hP))
                    if not half_major:
                        last = k == len(prs) - 1
                        for h in range(2):
                            nc.tensor.matmul(
                                sc[:, 512 * h : 512 * (h + 1)],
                                w2p[:, slot, :, :],
                                hP[:, :, 512 * h : 512 * (h + 1)],
                                start=False, stop=last, perf_mode=DR,
                            )
                    if k == 0 and after_first_pair is not None:
                        after_first_pair()
                if half_major:
                    # all h=0 matmuls first so bank h0 stops early (tail chunk)
                    for h in range(2):
                        sl = slice(512 * h, 512 * (h + 1))
                        for g, hT in enumerate(hTs):
                            nc.tensor.matmul(
                                sc[:, sl], wpk[:, 128 * g : 128 * (g + 1)],
                                hT[:, sl], start=(g == 0), stop=False,
                            )
                        for k, (slot, hP) in enumerate(hPs):
                            nc.tensor.matmul(
                                sc[:, sl], w2p[:, slot, :, :], hP[:, :, sl],
                                start=False, stop=(k == len(hPs) - 1), perf_mode=DR,
                            )
                scs[t] = sc

            def emit_exp_aw(t):
                u, s = t // 2, t % 2
                if s == 0:
                    atts[u] = apool.tile([128, 2, BC], FP8, tag="att", name=f"att{u}")
                    aws[u] = awpool.tile([128, 2, BC], FP8, tag="aw", name=f"aw{u}")
                att_p, aw_p = atts[u], aws[u]
                nc.scalar.activation(att_p[:, s, :], scs[t][:], AF.Exp, bias=shift_c)
                for h in range(2):
                    sl = slice(512 * h, 512 * (h + 1))
                    nc.gpsimd.tensor_tensor(
                        aw_p[:, s, sl], att_p[:, s, sl], ums[u][:, s, sl], ALU.mult
                    )
                scs[t] = None

            def emit_user(u, halves=(0, 1), quarters=(0, 1)):
                att_p, aw_p = atts[u], aws[u]
                for h in halves:
                    if dns[h] is None:
                        dns[h] = pstmp.tile([32, 512], F32, tag="tmp", name=f"dn{h}")
                    for k in quarters:
                        qs = slice(256 * k, 256 * (k + 1))
                        gs = slice(512 * h + 256 * k, 512 * h + 256 * (k + 1))
                        for s in range(2):
                            nc.tensor.matmul(
                                dns[h][0:1, qs], ones32[:, 0, 0:1], att_p[:, s, gs],
                                start=(u == 0 and s == 0),
                                stop=(u == NT // 2 - 1 and s == 1),
                                skip_group_check=True,
                            )
                        nc.tensor.matmul(
                            sus[h][:, qs], e_r8[:, 2 * u : 2 * u + 2, :], aw_p[:, :, gs],
                            start=(u == 0), stop=(u == NT // 2 - 1),
                            perf_mode=DR, skip_group_check=True,
                        )

            for t in range(NT):
                cb = (lambda tt=t: emit_exp_aw(tt - 1)) if t >= 1 else None
                emit_chunk(t, after_first_pair=cb)
                if t == 3:
                    emit_er_setup()
                if t == 4:
                    emit_ecT()
                if t == 5:
                    emit_user(0)
                    emit_user(1)
                if t == 6:
                    emit_user(2)

            # tail: chunk 7 exp/aw/user per half (aw on DVE to skip the slower
            # Pool op on the critical path), then a quartered finale ladder.
            uL = NT // 2 - 1
            att_l, aw_l = atts[uL], aws[uL]
            for h in range(2):
                for k in range(2):
                    gs = slice(512 * h + 256 * k, 512 * h + 256 * (k + 1))
                    nc.scalar.activation(att_l[:, 1, gs], scs[NT - 1][:, gs], AF.Exp,
                                         bias=shift_c)
                    nc.vector.tensor_tensor(aw_l[:, 1, gs], att_l[:, 1, gs],
                                            ums[uL][:, 1, gs], ALU.mult)
                    emit_user(uL, halves=(h,), quarters=(k,))

            # ---------------- finale: normalize + MLP (4 blocks of 256) -----
            o_sb = fpool.tile([1, BC], F32, tag="o")
            NQ = 4
            lns, recip, bc_sb, u_sb, h1s, h2s = {}, {}, {}, {}, {}, {}
            ps1s, ps2s, ps3s = {}, {}, {}

            def qsl(q):  # slice within the half's 512 columns
                return slice(256 * (q % 2), 256 * (q % 2) + 256)

            # reciprocal of the denominator without Ln/Exp (avoids act-table
            # swaps): row + pad-correction -> fast approx reciprocal (custom
            # DVE, ~18 bits) -> fp32 broadcast matmul.
            r0s, rcs = {}, {}
            for q in range(NQ):
                h = q // 2
                r0s[q] = fpool.tile([1, 256], F32, tag=f"r0{q}", name=f"r0{q}")
                nc.vector.tensor_scalar(
                    r0s[q][:], dns[h][0:1, qsl(q)], lnb_c, None, ALU.add
                )
            for q in range(NQ):
                rcs[q] = fpool.tile([1, 256], F32, tag=f"rc{q}", name=f"rc{q}")
                nc.vector.reciprocal_approx_fast(out=rcs[q][:], in_=r0s[q][:])
            for q in range(NQ):
                psb = pstmp.tile([128, 512], F32, tag="tmp", name=f"psb{q}")
                nc.tensor.matmul(psb[:E, :256], ones_f32, rcs[q][:], start=True, stop=True)
                bc_sb[q] = fpool.tile([E, 256], F32, tag=f"bc{q}", name=f"bc{q}")
                nc.vector.tensor_copy(bc_sb[q][:], psb[:E, :256])
            for q in range(NQ):
                u_sb[q] = fpool.tile([E, 256], BF16, tag=f"u{q}", name=f"u{q}")
                nc.vector.tensor_mul(u_sb[q][:], sus[q // 2][:, qsl(q)], bc_sb[q][:])
            for q in range(NQ):
                gsl = slice(256 * q, 256 * (q + 1))
                ps1s[q] = pstmp.tile([128, 512], F32, tag="tmp", name=f"ps1_{q}")
                nc.tensor.matmul(ps1s[q][:D1, :256], wm1aT, e_cT[:, gsl], start=True, stop=False)
                nc.tensor.matmul(ps1s[q][:D1, :256], wm1bT, u_sb[q][:], start=False, stop=True)
            for q in range(NQ):
                h1s[q] = fpool.tile([D1, 256], BF16, tag=f"h1{q}", name=f"h1{q}")
                nc.scalar.activation(h1s[q][:], ps1s[q][:D1, :256], AF.Relu, bias=bm1_c)
            for q in range(NQ):
                ps2s[q] = pstmp.tile([128, 512], F32, tag="tmp", name=f"ps2_{q}")
                nc.tensor.matmul(ps2s[q][:D2, :256], wm2T, h1s[q][:], start=True, stop=True)
            for q in range(NQ):
                h2s[q] = fpool.tile([D2, 256], BF16, tag=f"h2{q}", name=f"h2{q}")
                nc.scalar.activation(h2s[q][:], ps2s[q][:D2, :256], AF.Relu, bias=bm2_c)
            for q in range(NQ):
                ps3s[q] = pstmp.tile([128, 512], F32, tag="tmp", name=f"ps3_{q}")
                nc.tensor.matmul(ps3s[q][:1, :256], wm3T, h2s[q][:], start=True, stop=True)
            for q in range(NQ):
                gsl = slice(256 * q, 256 * (q + 1))
                nc.vector.tensor_scalar(o_sb[:, gsl], ps3s[q][:1, :256], bm3_c, None, ALU.add)
                eng = nc.sync if q % 2 == 0 else nc.gpsimd
                eng.dma_start(out=io["out"][0:1, gsl], in_=o_sb[:, gsl])


def build_nc():
    nc = bacc.Bacc("TRN2", target_bir_lowering=False)
    io = {
        "cprep": nc.dram_tensor("cprep", [128, BC], BF16, kind="ExternalInput"),
        "cpack": nc.dram_tensor("cpack", [128, 260], F32, kind="ExternalInput"),
        "wpk": nc.dram_tensor("wpk", [128, 1764], BF16, kind="ExternalInput"),
        "w2p": nc.dram_tensor("w2p", [128, 4, 2, 128], FP8, kind="ExternalInput"),
        "weT": nc.dram_tensor("weT", [128, NT, E], BF16, kind="ExternalInput"),
        "ratedT": nc.dram_tensor("ratedT", [128, NT, I], FP8, kind="ExternalInput"),
        "weT8": nc.dram_tensor("weT8", [128, NT, E], FP8, kind="ExternalInput"),
        "candT": nc.dram_tensor("candT", [128, NT, BC], BF16, kind="ExternalInput"),
        "out": nc.dram_tensor("out", [1, BC], F32, kind="ExternalOutput"),
    }
    for u in range(4):
        io[f"um{u}"] = nc.dram_tensor(f"um{u}", [128, 2, BC], FP8, kind="ExternalInput")
    emit(nc, io)
    nc.compile()
    return nc


def host_prep(candidate_items, rated_items, user_matrix, We, be, Wa1, ba1, Wa2,
              ba2, Wm1, bm1, Wm2, bm2, Wm3, bm3):
    f = np.float32
    cand = np.asarray(candidate_items, f)
    rated = np.asarray(rated_items, f)
    um = np.asarray(user_matrix, f)
    We, be = np.asarray(We, f), np.asarray(be, f)
    Wa1, ba1 = np.asarray(Wa1, f), np.asarray(ba1, f)
    wa2 = np.asarray(Wa2, f)[0]
    Wm1, bm1 = np.asarray(Wm1, f), np.asarray(bm1, f)
    Wm2, bm2 = np.asarray(Wm2, f), np.asarray(bm2, f)
    Wm3, bm3 = np.asarray(Wm3, f), np.asarray(bm3, f)

    W1c, W1r = Wa1[:, :E], Wa1[:, E:]
    aw2 = np.abs(wa2)
    sg = np.where(wa2 >= 0, 1.0, -1.0).astype(f)

    # attention projections with |wa2| folded in
    cp = (cand @ (W1c @ We).T + (W1c @ be)) * aw2[None, :]  # [B, ATT]
    e_r_h = rated @ We.T + be
    rp = (e_r_h @ W1r.T + ba1) * aw2[None, :]  # [I, ATT]
    rp_cols = rp.reshape(125, 8, ATT).transpose(1, 2, 0).reshape(128, 125)

    cpack = np.zeros((128, 260), f)
    cpack[:, 0:125] = rp_cols
    cpack[0:D1, 125] = bm1
    cpack[0:D2, 126] = bm2
    cpack[0, 127] = bm3[0]
    cpack[0:64, 128:192] = np.eye(64, dtype=f)
    cpack[:, 192] = SHIFT
    cpack[0, 193] = -1.0
    cpack[64, 193] = LNBIAS
    cpack[0, 194:258] = 1.0

    def sign_block(g):
        blk = np.zeros((128, 128), f)
        for il in range(8):
            blk[16 * il : 16 * il + ATT, 8 * g + il] = sg
        return blk

    wpk = np.zeros((128, 1764), BF)
    for g in range(12):
        wpk[:, 128 * g : 128 * (g + 1)] = sign_block(g).astype(BF)
    wpk[0:E, 1536:1600] = Wm1[:, :E].T.astype(BF)
    wpk[0:E, 1600:1664] = Wm1[:, E:].T.astype(BF)
    wpk[0:D1, 1664:1696] = Wm2.T.astype(BF)
    wpk[0:D2, 1696] = Wm3[0].astype(BF)
    wpk[0, 1697:1761] = 1.0

    w2p = np.zeros((128, 4, 2, 128), E4)
    for p, pair in enumerate(PAIR_SLOTS):
        for s, g in enumerate(pair):
            w2p[:, p, s, :] = sign_block(g).astype(E4)

    # weT [128, NT, E] with be folded via the ones row (D-row 1000)
    weT = np.zeros((128, NT, E), BF)
    WeT = We.T  # [D, E]
    for c in range(NT):
        r0 = 128 * c
        nrow = min(128, D - r0)
        if nrow > 0:
            weT[:nrow, c, :] = WeT[r0 : r0 + nrow, :].astype(BF)
    weT[104, 7, :] = be.astype(BF)  # D-row 1000 = bias row

    def stage_T(x, ncols, dt=BF):  # [N, D] -> [128, NT, ncols], ones at D-row 1000
        out = np.zeros((128, NT, ncols), dt)
        xT = x.T  # [D, N]
        for c in range(NT):
            r0 = 128 * c
            nrow = min(128, D - r0)
            if nrow > 0:
                out[:nrow, c, :] = xT[r0 : r0 + nrow, :].astype(dt)
        out[104, 7, :] = 1.0
        return out

    ratedT = stage_T(rated, I, E4)
    weT8 = np.zeros((128, NT, E), E4)
    weT8[:] = weT.astype(E4)

    umT = np.zeros((1024, B), E4)  # [i, b] zero-padded
    umT[:I] = um.T.astype(E4)

    shared = {
        "cpack": cpack,
        "wpk": wpk,
        "w2p": w2p,
        "weT": weT.reshape(128, NT * E),
        "weT8": weT8.reshape(128, NT * E),
        "ratedT": ratedT.reshape(128, NT * I),
    }
    in_maps = []
    for k in range(NCORES):
        m = dict(shared)
        bsl = slice(BC * k, BC * (k + 1))
        m["candT"] = np.ascontiguousarray(stage_T(cand[bsl], BC).reshape(128, NT * BC))
        cpk = cp[bsl]  # [BC, ATT]
        m["cprep"] = np.ascontiguousarray(cpk.T[np.arange(128) % ATT, :]).astype(BF)
        for u in range(4):
            blk = umT[256 * u : 256 * (u + 1), bsl]  # [256, BC]
            m[f"um{u}"] = np.ascontiguousarray(blk.reshape(2, 128, BC).transpose(1, 0, 2))
        in_maps.append(m)
    return in_maps


_NC_CACHE = {}


def _get_nc():
    if "nc" not in _NC_CACHE:
        _NC_CACHE["nc"] = build_nc()
    return _NC_CACHE["nc"]


def _install_ntff_hook():
    """Provide antenv.axon_hooks (absent in this image) so trace=True works."""
    import contextlib
    import ctypes
    import types

    if "antenv.axon_hooks" in sys.modules:
        return
    mod = types.ModuleType("antenv.axon_hooks")
    holder = {}
    mod.set_axon_ntff_profile_hook = lambda h: holder.__setitem__("h", h)
    mod.get_axon_ntff_profile_hook = lambda: holder.get("h")
    import antenv

    antenv.axon_hooks = mod
    sys.modules["antenv.axon_hooks"] = mod

    so_path = "/opt/axon/libaxon_pjrt.so"
    lib = ctypes.CDLL(so_path)
    if not hasattr(lib, "axon_start_nrt_profile"):
        return
    lib.axon_start_nrt_profile.argtypes = [ctypes.POINTER(ctypes.c_int64), ctypes.c_size_t]
    lib.axon_start_nrt_profile.restype = ctypes.c_int64
    lib.axon_stop_nrt_profile.argtypes = [ctypes.c_char_p]
    lib.axon_stop_nrt_profile.restype = ctypes.c_int64

    @contextlib.contextmanager
    def _hook(output_dir, device_ids):
        import jax

        jax.devices()
        if device_ids:
            ids = (ctypes.c_int64 * len(device_ids))(*device_ids)
            rc = lib.axon_start_nrt_profile(ids, len(device_ids))
        else:
            rc = lib.axon_start_nrt_profile(None, 0)
        if rc != 0:
            raise RuntimeError(f"axon_start_nrt_profile rc={rc}")
        try:
            yield
        finally:
            n = lib.axon_stop_nrt_profile(str(output_dir).encode())
            print(f"ntff profile: {n} file(s) written to {output_dir}", file=sys.stderr)

    mod.set_axon_ntff_profile_hook(_hook)


def run(inputs, trace=False, **kw):
    if trace:
        _install_ntff_hook()
    nc = _get_nc()
    in_maps = host_prep(**inputs)
    res = run_bass_kernel_spmd(nc, in_maps, list(range(NCORES)), trace=trace, **kw)
    out = np.concatenate(
        [np.asarray(res.results[k]["out"]).reshape(BC, 1) for k in range(NCORES)], axis=0
    ).astype(np.float32)
    return out, res


def kernel(**inputs):
    out, _ = run(inputs, trace=False)
    return out


# revision 26
# speedup vs baseline: 1.0614x; 1.0614x over previous
"""AttentionNCF Trainium2 kernel (SPMD over 8 NeuronCores, data-parallel over B).

Math (per batch row b, rated item i):
  e_c = cand @ We.T + be                  [B, E]
  e_r = rated @ We.T + be                 [I, E]
  scores[b,i] = sum_a wa2[a] * relu(cp[b,a] + rp[i,a])
              = sum_a sign(wa2[a]) * relu(|wa2[a]|*(cp[b,a] + rp[i,a]))
  att = softmax_i(scores); user_emb = (att*um) @ e_r
  out = MLP(concat[e_c, user_emb])

Device mapping (per core, BC=1024 rows of B):
  H layout: partitions = (i_local 8, a 16), free = b.  |wa2| is folded into
  cp/rp on host so formation is a plain relu and the score contraction
  stationary is the exact sign pattern (+-1).
  Per 16-group chunk: 12 groups formed on DVE (bf16, 4x mode) -> bf16
  matmuls; 4 groups formed on ScalarE (fp8e4 out) -> 2 fp8 DoubleRow pair
  matmuls (2 k-tiles per instruction).  att = exp(scores + SHIFT) in fp8;
  aw = att*um in fp8 on GPSIMD; user_emb/denominator accumulate via fp8
  DoubleRow matmuls over chunk pairs.
"""

import sys

import ml_dtypes
import numpy as np

sys.path.insert(0, "/opt/trn_rl_repo")

BF = ml_dtypes.bfloat16
E4 = ml_dtypes.float8_e4m3

import concourse.bass as bass
import concourse.mybir as mybir
import concourse.tile as tile
from concourse import bacc
from concourse.bass_utils import run_bass_kernel_spmd

F32 = mybir.dt.float32
BF16 = mybir.dt.bfloat16
FP8 = mybir.dt.float8e4
AF = mybir.ActivationFunctionType
ALU = mybir.AluOpType
DR = mybir.MatmulPerfMode.DoubleRow

B, I, D, E, ATT = 8192, 1000, 1000, 64, 16
D1, D2 = 64, 32
NCORES = 8
BC = B // NCORES  # 1024 batch rows per core
NT = 8  # i-chunks of 128 (7 full + 1 partial of 104)
NPAD = 24  # pad i rows in chunk 7
SHIFT = 2.5  # softmax-invariant shift so exp() fits fp8e4 comfortably
PAD_ATT = float(np.asarray(np.exp(SHIFT), dtype=E4))  # fp8-rounded exp(SHIFT)
LNBIAS = -NPAD * PAD_ATT  # subtract pad contribution from the denominator

# per-chunk group split: first NS groups on DVE (bf16), then NPAIR fp8 pairs
# formed on ScalarE.  chunk 7 has 13 groups: 9 singles + pairs (9,10),(11,12).
NS_FULL, NPAIR = 12, 2
PAIR_SLOTS = [(12, 13), (14, 15), (9, 10), (11, 12)]  # w2p bank layout


def _chunk_plan(t):
    if t < NT - 1:
        return 12, [(0, (12, 13)), (1, (14, 15))]
    return 9, [(2, (9, 10)), (3, (11, 12))]


def emit(nc, io):
    """Emit the kernel body. io maps tensor names to DRAM APs."""
    with tile.TileContext(nc) as tc:
        with (
            tc.tile_pool(name="const", bufs=1) as cpool,
            tc.tile_pool(name="inbig", bufs=1) as ipool,
            tc.tile_pool(name="stat", bufs=1) as spool,
            tc.tile_pool(name="hform", bufs=11) as hpool,
            tc.tile_pool(name="hform8", bufs=3) as h8pool,
            tc.tile_pool(name="att", bufs=3) as apool,
            tc.tile_pool(name="aw", bufs=3) as awpool,
            tc.tile_pool(name="fin", bufs=2) as fpool,
            tc.tile_pool(name="pstmp", bufs=2, space="PSUM") as pstmp,
            tc.tile_pool(name="pssc", bufs=2, space="PSUM") as pssc,
            tc.tile_pool(name="pssu", bufs=1, space="PSUM") as pssu,
        ):
            # -------- inputs to SBUF (single sync DMA queue, needed-first) --
            cpack = cpool.tile([128, 260], F32)
            nc.sync.dma_start(out=cpack[:], in_=io["cpack"][:])
            cprep = spool.tile([128, BC], BF16)
            nc.sync.dma_start(out=cprep[:], in_=io["cprep"][:])
            rp_cols = cpack[:, 0:125]
            bm1_c = cpack[0:D1, 125:126]
            bm2_c = cpack[0:D2, 126:127]
            bm3_c = cpack[0:1, 127:128]
            ident = cpack[0:64, 128:192]
            shift_c = cpack[:, 192:193]
            neg1_c = cpack[0:1, 193:194]
            lnb_c = cpack[64:65, 193:194]
            ones_f32 = cpack[0:1, 194:258]
            wpk = cpool.tile([128, 1764], BF16)
            nc.sync.dma_start(out=wpk[:, 0:256], in_=io["wpk"][:, 0:256])
            nc.sync.dma_start(out=wpk[:, 256:1764], in_=io["wpk"][:, 256:1764])
            wm1aT = wpk[0:E, 1536:1600]
            wm1bT = wpk[0:E, 1600:1664]
            wm2T = wpk[0:D1, 1664:1696]
            wm3T = wpk[0:D2, 1696:1697]
            ones_bf = wpk[0:1, 1697:1761]
            w2p = cpool.tile([128, 4, 2, 128], FP8)
            nc.sync.dma_start(out=w2p[:], in_=io["w2p"][:])
            weT = cpool.tile([128, NT, E], BF16)
            nc.sync.dma_start(out=weT[:], in_=io["weT"][:])
            weT8 = cpool.tile([128, NT, E], FP8)
            nc.sync.dma_start(out=weT8[:], in_=io["weT8"][:])
            rated8 = ipool.tile([128, NT, I], FP8)
            nc.sync.dma_start(out=rated8[:], in_=io["ratedT"][:])
            ums = []
            for u in range(4):
                um_t = ipool.tile([128, 2, BC], FP8, name=f"um{u}")
                ums.append(um_t)
            nc.sync.dma_start(out=ums[0][:], in_=io["um0"][:])
            cand = ipool.tile([128, NT, BC], BF16)
            nc.sync.dma_start(out=cand[:], in_=io["candT"][:])
            for u in range(1, 4):
                nc.sync.dma_start(out=ums[u][:], in_=io[f"um{u}"][:])

            ones32 = cpool.tile([128, 2, 32], FP8)
            nc.gpsimd.memset(ones32[:], 1.0)

            # act-table warmup: touch Relu/Exp/Ln before any real ACT work so
            # the table load(s) happen during the initial DMA wait, not on the
            # critical path.
            warm = cpool.tile([1, 4], F32)
            nc.vector.memset(warm[:], 1.0)
            wout = cpool.tile([1, 4], F32)
            nc.scalar.activation(wout[:], warm[:], AF.Relu)
            nc.scalar.activation(wout[:], warm[:], AF.Exp)

            e_cT = spool.tile([E, BC], BF16)
            e_r8 = spool.tile([128, NT, E], FP8)

            def emit_ecT():
                for h in range(2):
                    sl = slice(512 * h, 512 * (h + 1))
                    ps = pstmp.tile([128, 512], F32, tag="tmp", name=f"psec{h}")
                    for c in range(NT):
                        nc.tensor.matmul(
                            ps[:E, :], weT[:, c, :], cand[:, c, sl],
                            start=(c == 0), stop=(c == NT - 1),
                        )
                    nc.vector.tensor_copy(e_cT[:, sl], ps[:E, :])

            def emit_er_setup():
                e_rT = spool.tile([E, 1024], F32)
                nc.vector.memset(e_rT[:, I:1024], 0.0)
                for h, n0, nw in ((0, 0, 500), (1, 500, 500)):
                    ps = pstmp.tile([128, 512], F32, tag="tmp")
                    for c2 in range(NT // 2):
                        nc.tensor.matmul(
                            ps[:E, :nw],
                            weT8[:, 2 * c2 : 2 * c2 + 2, :],
                            rated8[:, 2 * c2 : 2 * c2 + 2, n0 : n0 + nw],
                            start=(c2 == 0), stop=(c2 == NT // 2 - 1), perf_mode=DR,
                        )
                    nc.vector.tensor_copy(e_rT[:, n0 : n0 + nw], ps[:E, :nw])
                for c in range(NT):
                    ps = pstmp.tile([128, 512], F32, tag="tmp")
                    nc.tensor.transpose(ps[:, :E], e_rT[:, 128 * c : 128 * (c + 1)], ident)
                    nc.vector.tensor_copy(e_r8[:, c, :], ps[:, :E])

            # ---------------- main loop over i-chunks ----------------------
            su0 = pssu.tile([64, 512], F32)  # user_emb accumulators
            su1 = pssu.tile([64, 512], F32)
            sus = (su0, su1)
            dns = [None, None]  # denominator accumulators (pstmp bufs, DR dst 0)
            scs = [None] * NT
            atts = [None] * (NT // 2)  # per chunk pair
            aws = [None] * (NT // 2)

            def emit_chunk(t, after_first_pair=None, half_major=False):
                ns, prs = _chunk_plan(t)
                sc = pssc.tile([128, 1024], F32, tag="sc")
                hTs, hPs = [], []
                for g in range(ns):
                    G = 16 * t + g
                    hT = hpool.tile([128, BC], BF16, tag="h", name=f"hT{t}_{g}")
                    nc.vector.tensor_scalar(
                        hT[:], cprep[:], rp_cols[:, G : G + 1], 0.0, ALU.add, ALU.max
                    )
                    hTs.append(hT)
                    if not half_major:
                        for h in range(2):
                            nc.tensor.matmul(
                                sc[:, 512 * h : 512 * (h + 1)],
                                wpk[:, 128 * g : 128 * (g + 1)],
                                hT[:, 512 * h : 512 * (h + 1)],
                                start=(g == 0), stop=False,
                            )
                for k, (slot, (gA, gB)) in enumerate(prs):
                    hP = h8pool.tile([128, 2, BC], FP8, tag="h8", name=f"hP{t}_{k}")
                    for s, g in ((0, gA), (1, gB)):
                        G = 16 * t + g
                        nc.scalar.activation(
                            hP[:, s, :], cprep[:], AF.Relu, bias=rp_cols[:, G : G + 1]
                        )
                    hPs.append((slot, hP))
                    if not half_major:
                        last = k == len(prs) - 1
                        for h in range(2):
                            nc.tensor.matmul(
                                sc[:, 512 * h : 512 * (h + 1)],
                                w2p[:, slot, :, :],
                                hP[:, :, 512 * h : 512 * (h + 1)],
                                start=False, stop=last, perf_mode=DR,
                            )
                    if k == 0 and after_first_pair is not None:
                        after_first_pair()
                if half_major:
                    # all h=0 matmuls first so bank h0 stops early (tail chunk)
                    for h in range(2):
                        sl = slice(512 * h, 512 * (h + 1))
                        for g, hT in enumerate(hTs):
                            nc.tensor.matmul(
                                sc[:, sl], wpk[:, 128 * g : 128 * (g + 1)],
                                hT[:, sl], start=(g == 0), stop=False,
                            )
                        for k, (slot, hP) in enumerate(hPs):
                            nc.tensor.matmul(
                                sc[:, sl], w2p[:, slot, :, :], hP[:, :, sl],
                                start=False, stop=(k == len(hPs) - 1), perf_mode=DR,
                            )
                scs[t] = sc

            def emit_exp_aw(t):
                u, s = t // 2, t % 2
                if s == 0:
                    atts[u] = apool.tile([128, 2, BC], FP8, tag="att", name=f"att{u}")
                    aws[u] = awpool.tile([128, 2, BC], FP8, tag="aw", name=f"aw{u}")
                att_p, aw_p = atts[u], aws[u]
                nc.scalar.activation(att_p[:, s, :], scs[t][:], AF.Exp, bias=shift_c)
                for h in range(2):
                    sl = slice(512 * h, 512 * (h + 1))
                    nc.gpsimd.tensor_tensor(
                        aw_p[:, s, sl], att_p[:, s, sl], ums[u][:, s, sl], ALU.mult
                    )
                scs[t] = None

            def emit_user(u, halves=(0, 1)):
                att_p, aw_p = atts[u], aws[u]
                for h in halves:
                    if dns[h] is None:
                        dns[h] = pstmp.tile([32, 512], F32, tag="tmp", name=f"dn{h}")
                    sl = slice(512 * h, 512 * (h + 1))
                    nc.tensor.matmul(
                        dns[h][:, :], ones32[:], att_p[:, :, sl],
                        start=(u == 0), stop=(u == NT // 2 - 1),
                        perf_mode=DR, skip_group_check=True,
                    )
                    nc.tensor.matmul(
                        sus[h][:, :], e_r8[:, 2 * u : 2 * u + 2, :], aw_p[:, :, sl],
                        start=(u == 0), stop=(u == NT // 2 - 1),
                        perf_mode=DR, skip_group_check=True,
                    )

            for t in range(NT):
                cb = (lambda tt=t: emit_exp_aw(tt - 1)) if t >= 1 else None
                emit_chunk(t, after_first_pair=cb)
                if t == 3:
                    emit_er_setup()
                if t == 4:
                    emit_ecT()
                if t == 5:
                    emit_user(0)
                    emit_user(1)
                if t == 6:
                    emit_user(2)

            # tail: chunk 7 exp/aw/user per half (aw on DVE to skip the slower
            # Pool op on the critical path), then a quartered finale ladder.
            uL = NT // 2 - 1
            att_l, aw_l = atts[uL], aws[uL]
            for h in range(2):
                sl = slice(512 * h, 512 * (h + 1))
                nc.scalar.activation(att_l[:, 1, sl], scs[NT - 1][:, sl], AF.Exp,
                                     bias=shift_c)
                nc.vector.tensor_tensor(aw_l[:, 1, sl], att_l[:, 1, sl],
                                        ums[uL][:, 1, sl], ALU.mult)
                emit_user(uL, halves=(h,))

            # ---------------- finale: normalize + MLP (4 blocks of 256) -----
            o_sb = fpool.tile([1, BC], F32, tag="o")
            NQ = 4
            lns, recip, bc_sb, u_sb, h1s, h2s = {}, {}, {}, {}, {}, {}
            ps1s, ps2s, ps3s = {}, {}, {}

            def qsl(q):  # slice within the half's 512 columns
                return slice(256 * (q % 2), 256 * (q % 2) + 256)

            # reciprocal of the denominator without Ln/Exp (avoids act-table
            # swaps): row + pad-correction -> fast approx reciprocal (custom
            # DVE, ~18 bits) -> fp32 broadcast matmul.
            r0s, rcs = {}, {}
            for q in range(NQ):
                h = q // 2
                r0s[q] = fpool.tile([1, 256], F32, tag=f"r0{q}", name=f"r0{q}")
                nc.vector.tensor_scalar(
                    r0s[q][:], dns[h][0:1, qsl(q)], lnb_c, None, ALU.add
                )
            for q in range(NQ):
                rcs[q] = fpool.tile([1, 256], F32, tag=f"rc{q}", name=f"rc{q}")
                nc.vector.reciprocal_approx_fast(out=rcs[q][:], in_=r0s[q][:])
            for q in range(NQ):
                psb = pstmp.tile([128, 512], F32, tag="tmp", name=f"psb{q}")
                nc.tensor.matmul(psb[:E, :256], ones_f32, rcs[q][:], start=True, stop=True)
                bc_sb[q] = fpool.tile([E, 256], F32, tag=f"bc{q}", name=f"bc{q}")
                nc.vector.tensor_copy(bc_sb[q][:], psb[:E, :256])
            for q in range(NQ):
                u_sb[q] = fpool.tile([E, 256], BF16, tag=f"u{q}", name=f"u{q}")
                nc.vector.tensor_mul(u_sb[q][:], sus[q // 2][:, qsl(q)], bc_sb[q][:])
            for q in range(NQ):
                gsl = slice(256 * q, 256 * (q + 1))
                ps1s[q] = pstmp.tile([128, 512], F32, tag="tmp", name=f"ps1_{q}")
                nc.tensor.matmul(ps1s[q][:D1, :256], wm1aT, e_cT[:, gsl], start=True, stop=False)
                nc.tensor.matmul(ps1s[q][:D1, :256], wm1bT, u_sb[q][:], start=False, stop=True)
            for q in range(NQ):
                h1s[q] = fpool.tile([D1, 256], BF16, tag=f"h1{q}", name=f"h1{q}")
                nc.scalar.activation(h1s[q][:], ps1s[q][:D1, :256], AF.Relu, bias=bm1_c)
            for q in range(NQ):
                ps2s[q] = pstmp.tile([128, 512], F32, tag="tmp", name=f"ps2_{q}")
                nc.tensor.matmul(ps2s[q][:D2, :256], wm2T, h1s[q][:], start=True, stop=True)
            for q in range(NQ):
                h2s[q] = fpool.tile([D2, 256], BF16, tag=f"h2{q}", name=f"h2{q}")
                nc.scalar.activation(h2s[q][:], ps2s[q][:D2, :256], AF.Relu, bias=bm2_c)
            for q in range(NQ):
                ps3s[q] = pstmp.tile([128, 512], F32, tag="tmp", name=f"ps3_{q}")
                nc.tensor.matmul(ps3s[q][:1, :256], wm3T, h2s[q][:], start=True, stop=True)
            for q in range(NQ):
                gsl = slice(256 * q, 256 * (q + 1))
                nc.vector.tensor_scalar(o_sb[:, gsl], ps3s[q][:1, :256], bm3_c, None, ALU.add)
                eng = nc.sync if q % 2 == 0 else nc.gpsimd
                eng.dma_start(out=io["out"][0:1, gsl], in_=o_sb[:, gsl])


def build_nc():
    nc = bacc.Bacc("TRN2", target_bir_lowering=False)
    io = {
        "cprep": nc.dram_tensor("cprep", [128, BC], BF16, kind="ExternalInput"),
        "cpack": nc.dram_tensor("cpack", [128, 260], F32, kind="ExternalInput"),
        "wpk": nc.dram_tensor("wpk", [128, 1764], BF16, kind="ExternalInput"),
        "w2p": nc.dram_tensor("w2p", [128, 4, 2, 128], FP8, kind="ExternalInput"),
        "weT": nc.dram_tensor("weT", [128, NT, E], BF16, kind="ExternalInput"),
        "ratedT": nc.dram_tensor("ratedT", [128, NT, I], FP8, kind="ExternalInput"),
        "weT8": nc.dram_tensor("weT8", [128, NT, E], FP8, kind="ExternalInput"),
        "candT": nc.dram_tensor("candT", [128, NT, BC], BF16, kind="ExternalInput"),
        "out": nc.dram_tensor("out", [1, BC], F32, kind="ExternalOutput"),
    }
    for u in range(4):
        io[f"um{u}"] = nc.dram_tensor(f"um{u}", [128, 2, BC], FP8, kind="ExternalInput")
    emit(nc, io)
    nc.compile()
    return nc


def host_prep(candidate_items, rated_items, user_matrix, We, be, Wa1, ba1, Wa2,
              ba2, Wm1, bm1, Wm2, bm2, Wm3, bm3):
    f = np.float32
    cand = np.asarray(candidate_items, f)
    rated = np.asarray(rated_items, f)
    um = np.asarray(user_matrix, f)
    We, be = np.asarray(We, f), np.asarray(be, f)
    Wa1, ba1 = np.asarray(Wa1, f), np.asarray(ba1, f)
    wa2 = np.asarray(Wa2, f)[0]
    Wm1, bm1 = np.asarray(Wm1, f), np.asarray(bm1, f)
    Wm2, bm2 = np.asarray(Wm2, f), np.asarray(bm2, f)
    Wm3, bm3 = np.asarray(Wm3, f), np.asarray(bm3, f)

    W1c, W1r = Wa1[:, :E], Wa1[:, E:]
    aw2 = np.abs(wa2)
    sg = np.where(wa2 >= 0, 1.0, -1.0).astype(f)

    # attention projections with |wa2| folded in
    cp = (cand @ (W1c @ We).T + (W1c @ be)) * aw2[None, :]  # [B, ATT]
    e_r_h = rated @ We.T + be
    rp = (e_r_h @ W1r.T + ba1) * aw2[None, :]  # [I, ATT]
    rp_cols = rp.reshape(125, 8, ATT).transpose(1, 2, 0).reshape(128, 125)

    cpack = np.zeros((128, 260), f)
    cpack[:, 0:125] = rp_cols
    cpack[0:D1, 125] = bm1
    cpack[0:D2, 126] = bm2
    cpack[0, 127] = bm3[0]
    cpack[0:64, 128:192] = np.eye(64, dtype=f)
    cpack[:, 192] = SHIFT
    cpack[0, 193] = -1.0
    cpack[64, 193] = LNBIAS
    cpack[0, 194:258] = 1.0

    def sign_block(g):
        blk = np.zeros((128, 128), f)
        for il in range(8):
            blk[16 * il : 16 * il + ATT, 8 * g + il] = sg
        return blk

    wpk = np.zeros((128, 1764), BF)
    for g in range(12):
        wpk[:, 128 * g : 128 * (g + 1)] = sign_block(g).astype(BF)
    wpk[0:E, 1536:1600] = Wm1[:, :E].T.astype(BF)
    wpk[0:E, 1600:1664] = Wm1[:, E:].T.astype(BF)
    wpk[0:D1, 1664:1696] = Wm2.T.astype(BF)
    wpk[0:D2, 1696] = Wm3[0].astype(BF)
    wpk[0, 1697:1761] = 1.0

    w2p = np.zeros((128, 4, 2, 128), E4)
    for p, pair in enumerate(PAIR_SLOTS):
        for s, g in enumerate(pair):
            w2p[:, p, s, :] = sign_block(g).astype(E4)

    # weT [128, NT, E] with be folded via the ones row (D-row 1000)
    weT = np.zeros((128, NT, E), BF)
    WeT = We.T  # [D, E]
    for c in range(NT):
        r0 = 128 * c
        nrow = min(128, D - r0)
        if nrow > 0:
            weT[:nrow, c, :] = WeT[r0 : r0 + nrow, :].astype(BF)
    weT[104, 7, :] = be.astype(BF)  # D-row 1000 = bias row

    def stage_T(x, ncols, dt=BF):  # [N, D] -> [128, NT, ncols], ones at D-row 1000
        out = np.zeros((128, NT, ncols), dt)
        xT = x.T  # [D, N]
        for c in range(NT):
            r0 = 128 * c
            nrow = min(128, D - r0)
            if nrow > 0:
                out[:nrow, c, :] = xT[r0 : r0 + nrow, :].astype(dt)
        out[104, 7, :] = 1.0
        return out

    ratedT = stage_T(rated, I, E4)
    weT8 = np.zeros((128, NT, E), E4)
    weT8[:] = weT.astype(E4)

    umT = np.zeros((1024, B), E4)  # [i, b] zero-padded
    umT[:I] = um.T.astype(E4)

    shared = {
        "cpack": cpack,
        "wpk": wpk,
        "w2p": w2p,
        "weT": weT.reshape(128, NT * E),
        "weT8": weT8.reshape(128, NT * E),
        "ratedT": ratedT.reshape(128, NT * I),
    }
    in_maps = []
    for k in range(NCORES):
        m = dict(shared)
        bsl = slice(BC * k, BC * (k + 1))
        m["candT"] = np.ascontiguousarray(stage_T(cand[bsl], BC).reshape(128, NT * BC))
        cpk = cp[bsl]  # [BC, ATT]
        m["cprep"] = np.ascontiguousarray(cpk.T[np.arange(128) % ATT, :]).astype(BF)
        for u in range(4):
            blk = umT[256 * u : 256 * (u + 1), bsl]  # [256, BC]
            m[f"um{u}"] = np.ascontiguousarray(blk.reshape(2, 128, BC).transpose(1, 0, 2))
        in_maps.append(m)
    return in_maps


_NC_CACHE = {}


def _get_nc():
    if "nc" not in _NC_CACHE:
        _NC_CACHE["nc"] = build_nc()
    return _NC_CACHE["nc"]


def _install_ntff_hook():
    """Provide antenv.axon_hooks (absent in this image) so trace=True works."""
    import contextlib
    import ctypes
    import types

    if "antenv.axon_hooks" in sys.modules:
        return
    mod = types.ModuleType("antenv.axon_hooks")
    holder = {}
    mod.set_axon_ntff_profile_hook = lambda h: holder.__setitem__("h", h)
    mod.get_axon_ntff_profile_hook = lambda: holder.get("h")
    import antenv

    antenv.axon_hooks = mod
    sys.modules["antenv.axon_hooks"] = mod

    so_path = "/opt/axon/libaxon_pjrt.so"
    lib = ctypes.CDLL(so_path)
    if not hasattr(lib, "axon_start_nrt_profile"):
        return
    lib.axon_start_nrt_profile.argtypes = [ctypes.POINTER(ctypes.c_int64), ctypes.c_size_t]
    lib.axon_start_nrt_profile.restype = ctypes.c_int64
    lib.axon_stop_nrt_profile.argtypes = [ctypes.c_char_p]
    lib.axon_stop_nrt_profile.restype = ctypes.c_int64

    @contextlib.contextmanager
    def _hook(output_dir, device_ids):
        import jax

        jax.devices()
        if device_ids:
            ids = (ctypes.c_int64 * len(device_ids))(*device_ids)
            rc = lib.axon_start_nrt_profile(ids, len(device_ids))
        else:
            rc = lib.axon_start_nrt_profile(None, 0)
        if rc != 0:
            raise RuntimeError(f"axon_start_nrt_profile rc={rc}")
        try:
            yield
        finally:
            n = lib.axon_stop_nrt_profile(str(output_dir).encode())
            print(f"ntff profile: {n} file(s) written to {output_dir}", file=sys.stderr)

    mod.set_axon_ntff_profile_hook(_hook)


def run(inputs, trace=False, **kw):
    if trace:
        _install_ntff_hook()
    nc = _get_nc()
    in_maps = host_prep(**inputs)
    res = run_bass_kernel_spmd(nc, in_maps, list(range(NCORES)), trace=trace, **kw)
    out = np.concatenate(
        [np.asarray(res.results[k]["out"]).reshape(BC, 1) for k in range(NCORES)], axis=0
    ).astype(np.float32)
    return out, res


def kernel(**inputs):
    out, _ = run(inputs, trace=False)
    return out
